# revision 1
# baseline (speedup 1.0000x reference)
"""Trainium2 Bass kernel for the GatedCRF 3D semseg loss.

Reformulation (validated vs reference to ~1e-6 rel):
  loss*denom = 2 * sum_{delta in HALF} sum_l E(l,d)*(y0[l]*y1[l+d] + y1[l]*y0[l+d])
             + sum_l G[l]*noob[l]
  E = exp(-0.5*((I[l+d]-I[l])/SIMG)^2 - 0.5*msq(delta))
  G = exp(-0.5*msq_center(l) - 0.5*(I[l]/SIMG)^2),  noob = # out-of-bounds offsets
where HALF is the 73 lexicographically-positive offsets of the 7x7x3 window
(center excluded); zero-padded y halos make the cross term vanish at every
volume boundary, so no per-offset masking is needed.

Sharding: the 73 offsets are strided across the 8 cores (SPMD program; each
core receives its own offset/bias tables as data; unused slots are disabled
with bias=-1e4 so exp()->0). Each core emits per-partition partial sums;
the host sums the 8 [128, NCOLS] partials and divides by N*H*W*D.

On-chip layout: partition p = 16*h_blk + w_blk is a (4h x 8w) spatial block;
per-partition storage keeps radius halos on all three axes (host pre-packs,
one DMA per volume). Window shifts become free-dim offsets loaded per slot
into Pool/DVE registers (values_load) and applied via register APs. The ISA
limits register APs to 2 free dims, so reads use [h_in] x [flat (w,d)-row]
patterns that include the d-halo columns; those columns hold y=0 on the
A side, so their garbage contributions vanish. y1/y0 are interleaved per
w-row so a single fused tensor_tensor_reduce per slot accumulates both
cross terms.
"""

import numpy as np

# problem constants (hardcoded per contract)
H, W, D = 64, 64, 32
SXY, SIMG = 5.0, 0.1
RH, RW, RD = 3, 3, 1
NCORES = 8
NSLOTS = 10                      # offset slots per core (73 = 7*9 + 1*10 -> pad)
BH, BW = 4, 8                    # central block per partition
NHB, NWB = H // BH, W // BW      # 16 x 8 blocks = 128 partitions
SH, SW, SD = BH + 2 * RH, BW + 2 * RW, D + 2 * RD   # 10, 14, 34 stored
FREE = SH * SW * SD              # 4760 stored elems per partition (J)
ROW = BW * SD                    # 272: fused (w,d) run per h_in (J side)
YROW = 2 * ROW                   # 544: fused (w,c,d) run per h_in (y side)
CEN = BH * BW * D                # 1024 central elems per partition
NCOLS = NSLOTS + 2               # acc columns: 1 per slot + G + spare
NMETA = CEN + 3 * NSLOTS         # meta: t3 | bias | joff-bits | yoff-bits
SQRT_HALF_OVER_SIG = float(np.sqrt(0.5) / SIMG)      # sqrt(50)
NEG = -1.0e4
DENOM = float(H * W * D)


def _half_offsets():
    offs = []
    for dh in range(0, RH + 1):
        for dw in range(-RW, RW + 1):
            for dd in range(-RD, RD + 1):
                if (dh > 0) or (dh == 0 and dw > 0) or (dh == 0 and dw == 0 and dd > 0):
                    offs.append((dh, dw, dd))
    assert len(offs) == 73
    return offs


def _pack_full(v):
    """(H, W, D) -> [128, SH, SW, SD]: per-partition block + halos, zero-padded."""
    vp = np.pad(v, ((RH, RH), (RW, RW), (RD, RD))).astype(np.float32)
    out = np.empty((128, SH, SW, SD), np.float32)
    for hb in range(NHB):
        for wb in range(NWB):
            out[hb * NWB + wb] = vp[hb * BH:hb * BH + SH, wb * BW:wb * BW + SW, :]
    return out


def _pack_blocks(v):
    """(H, W, D) -> [128, BH, BW, D] central-only block packing."""
    out = np.empty((128, BH, BW, D), np.float32)
    for hb in range(NHB):
        for wb in range(NWB):
            out[hb * NWB + wb] = v[hb * BH:(hb + 1) * BH, wb * BW:(wb + 1) * BW, :]
    return out


def _build_nc():
    import concourse.bass as bass
    import concourse.bacc as bacc
    import concourse.mybir as mybir
    from concourse.tile import TileContext

    f32, i32 = mybir.dt.float32, mybir.dt.int32
    AF = mybir.ActivationFunctionType
    OP = mybir.AluOpType
    ET = mybir.EngineType

    nc = bacc.Bacc("TRN2", target_bir_lowering=False, debug=False)
    vJ = nc.dram_tensor("vJ", [128, FREE], f32, kind="ExternalInput")
    # vy: y1/y0 interleaved per w-row: (SH, SW, 2, SD) flattened; [..,0,:]=y1
    vy = nc.dram_tensor("vy", [128, 2 * FREE], f32, kind="ExternalInput")
    meta = nc.dram_tensor("meta", [128, NMETA], f32, kind="ExternalInput")
    out = nc.dram_tensor("out", [128, NCOLS], f32, kind="ExternalOutput")

    with TileContext(nc) as tc:
        with tc.tile_pool(name="pers", bufs=1) as pers, \
             tc.tile_pool(name="wk", bufs=3) as wk, \
             tc.tile_pool(name="dpool", bufs=NSLOTS + 1) as dpool, \
             tc.tile_pool(name="jkpool", bufs=1) as jkpool, \
             tc.tile_pool(name="gpool", bufs=1) as gpool:
            # +PAD slack columns: worst-case shifted rows overrun the last
            # stored element by up to 2; keep the overrun readable and zero.
            PAD = 8
            J = pers.tile([128, FREE + PAD], f32, tag="J")
            ypair = pers.tile([128, 2 * FREE + PAD], f32, tag="ypair")
            metatile = pers.tile([128, NMETA], f32, tag="meta")
            acc = pers.tile([128, NCOLS], f32, tag="acc")

            nc.vector.memset(acc[:], 0.0)
            nc.vector.memset(J[:, FREE:], 0.0)
            nc.vector.memset(ypair[:, 2 * FREE:], 0.0)
            nc.sync.dma_start(metatile[:], meta[:])
            nc.sync.dma_start(J[:, 0:FREE], vJ[:])
            nc.sync.dma_start(ypair[:, 0:2 * FREE], vy[:])
            Jv = J[:, 0:FREE].rearrange("p (a b c) -> p a b c",
                                        a=SH, b=SW, c=SD)
            ypv = ypair[:, 0:2 * FREE].rearrange("p (a b c d) -> p a b c d",
                                                 a=SH, b=SW, c=2, d=SD)

            t3tile = metatile[:, 0:CEN].rearrange(
                "p (a b c) -> p a b c", a=BH, b=BW, c=D)
            biastile = metatile[:, CEN:CEN + NSLOTS]
            JOFF_COL = CEN + NSLOTS
            YOFF_COL = CEN + 2 * NSLOTS

            # Static A-views: central h/w rows, FULL d (incl. halo cols).
            J_A = Jv[:, RH:RH + BH, RW:RW + BW, :].rearrange(
                "p a b c -> p a (b c)")                       # [128, BH, ROW]
            y1_A = ypv[:, RH:RH + BH, RW:RW + BW, 0, :]       # [128, BH, BW, SD]
            y0_A = ypv[:, RH:RH + BH, RW:RW + BW, 1, :]

            # dynamic B-view patterns: [partition, h_in, flat row]
            jpat = [[FREE + PAD, 128], [SW * SD, BH], [1, ROW]]
            ypat = [[2 * FREE + PAD, 128], [2 * SW * SD, BH], [1, YROW]]
            jmax = 2 * RH * SW * SD + 2 * RW * SD + 2 * RD
            ymax = 2 * (2 * RH * SW * SD + 2 * RW * SD) + 2 * RD

            for j in range(NSLOTS):
                joff = nc.values_load(
                    metatile[0:1, JOFF_COL + j:JOFF_COL + j + 1].bitcast(i32),
                    engines=(ET.Pool,),
                    min_val=0, max_val=jmax,
                    skip_runtime_bounds_check=True,
                )
                yoff = nc.values_load(
                    metatile[0:1, YOFF_COL + j:YOFF_COL + j + 1].bitcast(i32),
                    engines=(ET.DVE,),
                    min_val=0, max_val=ymax,
                    skip_runtime_bounds_check=True,
                )

                # Pool: d = J_B - J_A   (sole Pool op; single-reader ACT)
                dt_ = dpool.tile([128, BH, ROW], f32, tag="d")
                nc.gpsimd.tensor_tensor(dt_[:], bass.AP(J.tensor, joff, jpat),
                                        J_A, OP.subtract)
                # ACT: q = 0.5*(d/SIMG)^2 ; E = exp(-q + bias)
                qt = wk.tile([128, BH, ROW], f32, tag="q")
                nc.scalar.activation(qt[:], dt_[:], AF.Square,
                                     scale=SQRT_HALF_OVER_SIG)
                et = wk.tile([128, BH, ROW], f32, tag="e")
                nc.scalar.activation(et[:], qt[:], AF.Exp, scale=-1.0,
                                     bias=biastile[:, j:j + 1])
                # DVE: eyp rows = [E*y0 | E*y1] interleaved like vy's (y1|y0)
                eyp = wk.tile([128, BH, BW, 2, SD], f32, tag="eyp")
                etv = et[:].rearrange("p a (b c) -> p a b c", b=BW, c=SD)
                nc.vector.tensor_tensor(eyp[:, :, :, 0, :], etv, y0_A, OP.mult)
                nc.vector.tensor_tensor(eyp[:, :, :, 1, :], etv, y1_A, OP.mult)
                wt = jkpool.tile([128, BH, BW, 2, SD], f32, tag="w")
                nc.vector.tensor_tensor(
                    wt[:].rearrange("p a b c d -> p a (b c d)"),
                    eyp[:].rearrange("p a b c d -> p a (b c d)"),
                    bass.AP(ypair.tensor, yoff, ypat), OP.mult)
                jk = jkpool.tile([128, BH, BW, 2, SD], f32, tag="jk")
                nc.scalar.activation(
                    jk[:].rearrange("p a b c d -> p a (b c d)"),
                    wt[:].rearrange("p a b c d -> p a (b c d)"),
                    AF.Identity, scale=2.0,
                    accum_out=acc[:, j:j + 1])

            # G-pass (out-of-bounds kernel mass); gated per-core via t3 data
            qg = gpool.tile([128, BH, BW, D], f32, tag="qg")
            nc.scalar.activation(qg[:], Jv[:, RH:RH + BH, RW:RW + BW, RD:RD + D],
                                 AF.Square, scale=SQRT_HALF_OVER_SIG)
            ag = gpool.tile([128, BH, BW, D], f32, tag="ag")
            nc.vector.scalar_tensor_tensor(ag[:], qg[:], -1.0, t3tile[:],
                                           OP.mult, OP.add)
            eg = gpool.tile([128, BH, BW, D], f32, tag="eg")
            nc.scalar.activation(eg[:], ag[:], AF.Exp,
                                 accum_out=acc[:, NSLOTS:NSLOTS + 1])

            nc.sync.dma_start(out[:], acc[:])
    nc.compile()
    return nc


def _host_tables(sample, spacing):
    """Per-core offset/bias tables + core-7 t3 table."""
    sp = np.asarray(spacing, dtype=np.float64)[:, 0]
    half = _half_offsets()
    per_core = [half[i::NCORES] for i in range(NCORES)]  # core0: 10, rest 9

    a_j = RH * SW * SD + RW * SD + RD
    a_y = RH * 2 * SW * SD + RW * 2 * SD + RD
    joff_tabs, yoff_tabs, bias_tabs = [], [], []
    for core in range(NCORES):
        jt = np.full((1, NSLOTS), a_j, np.int32)     # pad: B==A, bias NEG -> E=0
        yt = np.full((1, NSLOTS), a_y, np.int32)
        bt = np.full((128, NSLOTS), NEG, np.float32)
        for j, (dh, dw, dd) in enumerate(per_core[core]):
            jt[0, j] = (RH + dh) * SW * SD + (RW + dw) * SD + (RD + dd)
            yt[0, j] = (RH + dh) * 2 * SW * SD + (RW + dw) * 2 * SD + (RD + dd)
            msq = ((sp[0] * dh) ** 2 + (sp[1] * dw) ** 2 + (sp[2] * dd) ** 2) / SXY ** 2
            bt[:, j] = -0.5 * msq
        joff_tabs.append(jt)
        yoff_tabs.append(yt)
        bias_tabs.append(bt)

    # t3 = ln(noob) - 0.5*msq_center (NEG where noob == 0); real only on core 7
    h = np.arange(H)[:, None, None]
    w = np.arange(W)[None, :, None]
    d = np.arange(D)[None, None, :]
    msq_c = ((sp[0] * h) ** 2 + (sp[1] * w) ** 2 + (sp[2] * d) ** 2) / SXY ** 2
    cnt = ((np.minimum(h, RH) + np.minimum(H - 1 - h, RH) + 1)
           * (np.minimum(w, RW) + np.minimum(W - 1 - w, RW) + 1)
           * (np.minimum(d, RD) + np.minimum(D - 1 - d, RD) + 1))
    noob = (2 * RH + 1) * (2 * RW + 1) * (2 * RD + 1) - cnt
    t3full = np.where(noob > 0, np.log(np.maximum(noob, 1)) - 0.5 * msq_c, NEG)
    t3_real = _pack_blocks(t3full.astype(np.float32))
    t3_off = np.full((128, BH, BW, D), NEG, np.float32)
    return joff_tabs, yoff_tabs, bias_tabs, t3_real, t3_off


def _host_inputs(y_hat_softmax, sample, spacing):
    y = np.asarray(y_hat_softmax, dtype=np.float32)[0]       # (2, H, W, D)
    I = np.asarray(sample, dtype=np.float32)[0, 0]           # (H, W, D)
    vJ = _pack_full(I).reshape(128, FREE)
    # interleave y1/y0 per w-row: [128, SH, SW, 2, SD] -> flat
    vy = np.stack([_pack_full(y[1]), _pack_full(y[0])],
                  axis=3).reshape(128, 2 * FREE)
    joff_tabs, yoff_tabs, bias_tabs, t3_real, t3_off = _host_tables(
        sample, spacing)
    in_maps = []
    for core in range(NCORES):
        meta = np.zeros((128, NMETA), np.float32)
        t3c = t3_real if core == NCORES - 1 else t3_off
        meta[:, 0:CEN] = t3c.reshape(128, CEN)
        meta[:, CEN:CEN + NSLOTS] = bias_tabs[core]
        meta[0:1, CEN + NSLOTS:CEN + 2 * NSLOTS] = joff_tabs[core].view(np.float32)
        meta[0:1, CEN + 2 * NSLOTS:] = yoff_tabs[core].view(np.float32)
        in_maps.append({"vJ": vJ, "vy": vy, "meta": meta})
    return in_maps


def kernel(y_hat_softmax, sample, spacing):
    from concourse.bass_utils import run_bass_kernel_spmd

    in_maps = _host_inputs(y_hat_softmax, sample, spacing)
    nc = _build_nc()
    res = run_bass_kernel_spmd(nc, in_maps, core_ids=list(range(NCORES)))
    total = sum(float(r["out"].astype(np.float64).sum()) for r in res.results)
    return np.array(total / DENOM, dtype=np.float32)


if __name__ == "__main__":
    rng = np.random.default_rng(0)
    logits = rng.standard_normal((1, 2, H, W, D)).astype(np.float32)
    e = np.exp(logits - logits.max(axis=1, keepdims=True))
    yh = (e / e.sum(axis=1, keepdims=True)).astype(np.float32)
    smp = rng.standard_normal((1, 1, H, W, D)).astype(np.float32)
    spc = rng.uniform(0.5, 2.0, (3, 1)).astype(np.float32)
    print(kernel(yh, smp, spc))



# revision 8
# speedup vs baseline: 2.0271x; 2.0271x over previous
"""Trainium2 Bass kernel for the GatedCRF 3D semseg loss.

Reformulation (validated vs reference to ~6e-7 rel in fp64):
With C=2 softmax channels, y0+y1=1. Let a = 1-2*y0, then per voxel-pair
  y0A*y1B + y1A*y0B = (1 - aA*aB)/2
so with E(l,delta) = exp(-0.5*((I[l+d]-I[l])/SIMG)^2 - 0.5*msq(delta)):
  loss*denom = sum_{d in HALF} [ sum_l E  -  sum_l E*aA*aB ] + G_total
where HALF is the 73 lexicographically-positive offsets of the 7x7x3
window and G_total is the out-of-bounds kernel mass
sum_l noob(l)*exp(-0.5*msq_c(l) - 0.5*(I_l/SIMG)^2).

Validity masking is data-driven: out-of-volume halo voxels carry
J = BIG (3e4) so any one-sided-OOB pair gets E = exp(-huge) = 0, and
both-OOB pairs have aA = aB = 1 (u=0 pad) so E - E*aA*aB cancels
exactly. No per-offset masks needed.

Per offset this needs only 5 elementwise passes of FD=1088:
  sub (d = J_B - J_A), Square, Exp (sum_l E rides on accum_out),
  v = E*a_B, tensor_tensor_reduce((v*a_A) -> accum sum_l E*aA*aB).

Layout: partition p = 16*h_blk + w_blk is a (4h x 8w) spatial block with
halos; only dh >= 0 is ever read (half-offsets) so the top h-halo is
trimmed (7 stored rows). J and a are stored fp16 as DUAL COPIES
[X | X<<1elem] so every dynamic window shift resolves to a 4B-aligned
base (joff even -> copy0, odd -> copy1 at joff-1), which keeps the
DVE/ACT 16-bit 2x perf modes engaged. Offsets/biases are per-core data
(SPMD program); unused slots get bias=-1e4 so E -> 0. The G-pass is
sharded: each core handles 1/8 of the central voxels via a per-core
register offset.
"""

import numpy as np

# problem constants (hardcoded per contract)
H, W, D = 64, 64, 32
SXY, SIMG = 5.0, 0.1
RH, RW, RD = 3, 3, 1
NCORES = 8
NSLOTS = 10
BH, BW = 4, 8                    # central block per partition
NHB, NWB = H // BH, W // BW      # 16 x 8 blocks = 128 partitions
SH = BH + RH                     # 7 stored h rows (top halo trimmed)
SW = BW + 2 * RW                 # 14
SD = D + 2 * RD                  # 34
FREE = SH * SW * SD              # 3332 stored elems per partition
ROW = BW * SD                    # 272: fused (w,d) run per h row
PAD = 8
JP = 2 * FREE + PAD              # dual-copy tile free length (6672, even)
CENT = RW * SD + RD              # 103 (odd) central base in copy-0
CENT1 = FREE + CENT - 1          # 3434 (even) central base in copy-1
NCOLS = NSLOTS + 2               # acc columns: per-slot sums | G | spare
NMETA = 128 + NSLOTS + NSLOTS + 1
SQ = float(np.sqrt(0.5) / SIMG)  # sqrt(50)
BIG = 28.0      # halo marker: q = (sqrt(50)*(BIG+-I))^2 stays finite in fp16
                # yet exp(-q) == 0 for any one-sided-OOB pair (q >= ~24000)
NEG = -1.0e4
DENOM = float(H * W * D)
OFF_MAX = FREE + (RH * SW * SD + 2 * RW * SD + 2 * RD)


def _half_offsets():
    offs = []
    for dh in range(0, RH + 1):
        for dw in range(-RW, RW + 1):
            for dd in range(-RD, RD + 1):
                if (dh > 0) or (dh == 0 and dw > 0) or (dh == 0 and dw == 0 and dd > 0):
                    offs.append((dh, dw, dd))
    assert len(offs) == 73
    return offs


def _per_core_slots():
    """73 half-offsets -> 8 cores x 10 slots (None = dead slot).
    dd==0 slots (copy-1 reads) first so they can start while copy-0 lands."""
    offs = _half_offsets()
    dd0 = [o for o in offs if o[2] == 0]      # 24 -> 3 per core
    ddx = [o for o in offs if o[2] != 0]      # 49 -> core0: 7, rest: 6
    cores = []
    xi = 0
    for c in range(NCORES):
        n = 7 if c == 0 else 6
        slots = dd0[3 * c:3 * c + 3] + ddx[xi:xi + n]
        xi += n
        slots += [None] * (NSLOTS - len(slots))
        cores.append(slots)
    assert xi == len(ddx)
    return cores


def _pack(v, pad_val):
    """(H, W, D) -> [128, FREE]: per-partition block + trimmed halos."""
    vp = np.pad(v.astype(np.float32), ((RH, RH), (RW, RW), (RD, RD)),
                constant_values=pad_val)
    out = np.empty((128, SH, SW, SD), np.float32)
    for hb in range(NHB):
        for wb in range(NWB):
            out[hb * NWB + wb] = vp[hb * BH + RH:hb * BH + RH + SH,
                                    wb * BW:wb * BW + SW, :]
    return out.reshape(128, FREE)


def _dual_f16(flat, tail):
    """[128, FREE] -> fp16 [128, 2*FREE] = [X | X shifted left one elem]."""
    x = flat.astype(np.float16)
    x1 = np.empty_like(x)
    x1[:, :-1] = x[:, 1:]
    x1[:, -1] = tail
    return np.concatenate([x, x1], axis=1)


def _build_nc():
    import concourse.bass as bass
    import concourse.bacc as bacc
    import concourse.mybir as mybir
    from concourse.tile import TileContext

    f32, f16, i32 = mybir.dt.float32, mybir.dt.float16, mybir.dt.int32
    AF = mybir.ActivationFunctionType
    OP = mybir.AluOpType
    ET = mybir.EngineType

    nc = bacc.Bacc("TRN2", target_bir_lowering=False, debug=False)
    jp = nc.dram_tensor("jp", [128, 2 * FREE], f16, kind="ExternalInput")
    apd = nc.dram_tensor("apd", [128, 2 * FREE], f16, kind="ExternalInput")
    meta = nc.dram_tensor("meta", [128, NMETA], f32, kind="ExternalInput")
    out = nc.dram_tensor("out", [128, NCOLS], f32, kind="ExternalOutput")

    bpat = [[JP, 128], [SW * SD, BH], [1, ROW]]

    with TileContext(nc) as tc:
        with tc.tile_pool(name="pers", bufs=1) as pers, \
             tc.tile_pool(name="dpool", bufs=3) as dpool, \
             tc.tile_pool(name="qpool", bufs=3) as qpool, \
             tc.tile_pool(name="epool", bufs=3) as epool, \
             tc.tile_pool(name="vpool", bufs=3) as vpool, \
             tc.tile_pool(name="spool", bufs=2) as spool, \
             tc.tile_pool(name="gpool", bufs=1) as gpool:
            J = pers.tile([128, JP], f16, tag="J")
            A = pers.tile([128, JP], f16, tag="A")
            metatile = pers.tile([128, NMETA], f32, tag="meta")
            acc = pers.tile([128, NCOLS], f32, tag="acc")

            nc.vector.memset(acc[:], 0.0)
            nc.vector.memset(J[:, 2 * FREE:], 0.0)
            nc.vector.memset(A[:, 2 * FREE:], 0.0)
            nc.sync.dma_start(metatile[:], meta[:])
            nc.sync.dma_start(J[:, 0:2 * FREE], jp[:])
            nc.sync.dma_start(A[:, 0:2 * FREE], apd[:])

            t3v = metatile[:, 0:128]
            biasv = metatile[:, 128:128 + NSLOTS]
            JOFF_COL = 128 + NSLOTS
            GOFF_COL = 128 + 2 * NSLOTS

            _, jvals = nc.values_load_multi_w_load_instructions(
                metatile[0:1, JOFF_COL:JOFF_COL + NSLOTS].bitcast(i32),
                engines=(ET.DVE,), min_val=0, max_val=OFF_MAX,
                skip_runtime_bounds_check=True)
            _, pvals = nc.values_load_multi_w_load_instructions(
                metatile[0:1, JOFF_COL:JOFF_COL + NSLOTS].bitcast(i32),
                engines=(ET.Pool,), min_val=0, max_val=OFF_MAX,
                skip_runtime_bounds_check=True)
            gval = nc.values_load(
                metatile[0:1, GOFF_COL:GOFF_COL + 1].bitcast(i32),
                engines=(ET.Activation,), min_val=0, max_val=OFF_MAX,
                skip_runtime_bounds_check=True)

            J_A = bass.AP(J.tensor, CENT1, bpat)
            a_A = bass.AP(A.tensor, CENT1, bpat)

            for j in range(NSLOTS):
                # sub and c-mult alternate Pool/DVE so each engine gets
                # one TT per slot; STT (with fused accum) always on DVE.
                dt = dpool.tile([128, BH, ROW], f16, tag="d")
                ct = vpool.tile([128, BH, ROW], f16, tag="c")
                if j % 2 == 0:
                    nc.gpsimd.tensor_tensor(
                        dt[:], bass.AP(J.tensor, pvals[j], bpat), J_A,
                        OP.subtract)
                    nc.vector.tensor_tensor(
                        ct[:], bass.AP(A.tensor, jvals[j], bpat), a_A,
                        OP.mult)
                else:
                    nc.vector.tensor_tensor(
                        dt[:], bass.AP(J.tensor, jvals[j], bpat), J_A,
                        OP.subtract)
                    nc.gpsimd.tensor_tensor(
                        ct[:], bass.AP(A.tensor, pvals[j], bpat), a_A,
                        OP.mult)
                qt = qpool.tile([128, BH, ROW], f16, tag="q")
                nc.scalar.activation(qt[:], dt[:], AF.Square, scale=SQ)
                et = epool.tile([128, BH, ROW], f16, tag="e")
                nc.scalar.activation(et[:], qt[:], AF.Exp, scale=-1.0,
                                     bias=biasv[:, j:j + 1])
                # st = (c - 1) * E ; accum col j = sum = E*aA*aB - E
                st = spool.tile([128, BH, ROW], f16, tag="s")
                nc.vector.scalar_tensor_tensor(
                    st[:], ct[:], 1.0, et[:], OP.subtract, OP.mult,
                    accum_out=acc[:, j:j + 1])

            # G-pass: 1/8 of central voxels per core (one h-row x 4 w x D)
            qg = gpool.tile([128, 4, D], f16, tag="qg")
            nc.scalar.activation(
                qg[:], bass.AP(J.tensor, gval, [[JP, 128], [SD, 4], [1, D]]),
                AF.Square, scale=SQ)
            ag = gpool.tile([128, 4 * D], f32, tag="ag")
            nc.vector.scalar_tensor_tensor(
                ag[:], qg[:].rearrange("p a b -> p (a b)"), -1.0, t3v,
                OP.mult, OP.add)
            eg = gpool.tile([128, 4 * D], f16, tag="eg")
            nc.scalar.activation(eg[:], ag[:], AF.Exp,
                                 accum_out=acc[:, NSLOTS:NSLOTS + 1])

            nc.sync.dma_start(out[:], acc[:])
    nc.compile()
    return nc


def _host_tables(sample, spacing):
    """Per-core meta arrays: t3 slice | bias | joff_eff | gjoff_eff."""
    sp = np.asarray(spacing, dtype=np.float64)[:, 0]
    cores = _per_core_slots()

    # t3 = ln(noob) - 0.5*msq_center (NEG where noob == 0), central packing
    h = np.arange(H)[:, None, None]
    w = np.arange(W)[None, :, None]
    d = np.arange(D)[None, None, :]
    msq_c = ((sp[0] * h) ** 2 + (sp[1] * w) ** 2 + (sp[2] * d) ** 2) / SXY ** 2
    cnt = ((np.minimum(h, RH) + np.minimum(H - 1 - h, RH) + 1)
           * (np.minimum(w, RW) + np.minimum(W - 1 - w, RW) + 1)
           * (np.minimum(d, RD) + np.minimum(D - 1 - d, RD) + 1))
    noob = (2 * RH + 1) * (2 * RW + 1) * (2 * RD + 1) - cnt
    t3full = np.where(noob > 0, np.log(np.maximum(noob, 1)) - 0.5 * msq_c, NEG)
    t3b = np.empty((128, BH, BW, D), np.float32)
    for hb in range(NHB):
        for wb in range(NWB):
            t3b[hb * NWB + wb] = t3full[hb * BH:(hb + 1) * BH,
                                        wb * BW:(wb + 1) * BW, :]

    metas = []
    for c in range(NCORES):
        m = np.zeros((128, NMETA), np.float32)
        # G slice: h-row c//2, w cols 4*(c%2)..+4
        m[:, 0:128] = t3b[:, c // 2, 4 * (c % 2):4 * (c % 2) + 4, :].reshape(128, 128)
        joffs = np.full(NSLOTS, CENT1, np.int32)
        bias = np.full((128, NSLOTS), NEG, np.float32)
        for j, o in enumerate(cores[c]):
            if o is None:
                continue
            dh, dw, dd = o
            j0 = dh * SW * SD + (RW + dw) * SD + (RD + dd)
            joffs[j] = j0 if j0 % 2 == 0 else FREE + j0 - 1
            msq = ((sp[0] * dh) ** 2 + (sp[1] * dw) ** 2
                   + (sp[2] * dd) ** 2) / SXY ** 2
            bias[:, j] = -0.5 * msq
        m[:, 128:128 + NSLOTS] = bias
        m[0, 128 + NSLOTS:128 + 2 * NSLOTS] = joffs.view(np.float32)
        gj = (c // 2) * SW * SD + (RW + 4 * (c % 2)) * SD + RD
        m[0, 128 + 2 * NSLOTS] = np.int32(FREE + gj - 1).view(np.float32)
        metas.append(m)
    return metas


def _host_inputs(y_hat_softmax, sample, spacing):
    y0 = np.asarray(y_hat_softmax, dtype=np.float32)[0, 0]
    I = np.asarray(sample, dtype=np.float32)[0, 0]
    jp = _dual_f16(_pack(I, BIG), 0.0)
    apd = _dual_f16(_pack(1.0 - 2.0 * y0, 1.0), 1.0)
    metas = _host_tables(sample, spacing)
    return [{"jp": jp, "apd": apd, "meta": metas[c]} for c in range(NCORES)]


def kernel(y_hat_softmax, sample, spacing):
    from concourse.bass_utils import run_bass_kernel_spmd

    in_maps = _host_inputs(y_hat_softmax, sample, spacing)
    nc = _build_nc()
    res = run_bass_kernel_spmd(nc, in_maps, core_ids=list(range(NCORES)))
    total = 0.0
    for r in res.results:
        o = r["out"].astype(np.float64)
        total += -o[:, 0:NSLOTS].sum() + o[:, NSLOTS].sum()
    return np.array(total / DENOM, dtype=np.float32)


if __name__ == "__main__":
    rng = np.random.default_rng(0)
    logits = rng.standard_normal((1, 2, H, W, D)).astype(np.float32)
    e = np.exp(logits - logits.max(axis=1, keepdims=True))
    yh = (e / e.sum(axis=1, keepdims=True)).astype(np.float32)
    smp = rng.standard_normal((1, 1, H, W, D)).astype(np.float32)
    spc = rng.uniform(0.5, 2.0, (3, 1)).astype(np.float32)
    print(kernel(yh, smp, spc))


# revision 15
# speedup vs baseline: 2.3422x; 1.1554x over previous
"""Trainium2 Bass kernel for the GatedCRF 3D semseg loss.

Reformulation (validated vs reference to ~6e-7 rel in fp64):
With C=2 softmax channels, y0+y1=1. Let a = 1-2*y0, then per voxel-pair
  y0A*y1B + y1A*y0B = (1 - aA*aB)/2
so with E(l,delta) = exp(-0.5*((I[l+d]-I[l])/SIMG)^2 - 0.5*msq(delta)):
  loss*denom = sum_{d in HALF} [ sum_l E  -  sum_l E*aA*aB ] + G_total
where HALF is the 73 lexicographically-positive offsets of the 7x7x3
window and G_total is the out-of-bounds kernel mass
sum_l noob(l)*exp(-0.5*msq_c(l) - 0.5*(I_l/SIMG)^2).

Validity masking is data-driven: out-of-volume halo voxels carry J = BIG
so any one-sided-OOB pair gets E = exp(-huge) = 0, and both-OOB pairs
have aA = aB = 1 (u=0 pad) so E - E*aA*aB cancels exactly.

sum_l E rides free on the Exp's accum_out. The product side factors as
sum_l aA * Q(l) with Q = sum_d E_d * aB_d accumulated across slots by
in-place DVE adds, so each offset needs only {sub, Square, Exp, v-mult,
Q+=v} and the whole product reduction is ONE final STT-with-accum.

Engine notes (measured): GPSIMD shares its SBUF port with the DVE --
running it concurrently slows DVE 3.5x, so everything stays on DVE+ACT.
ACT is 1x-rate ((FD+352)/1.2GHz) regardless of dtype; DVE fp16 TT with
step-1 4B-aligned operands runs 2x ((FD/2+151)/0.96GHz). J and a are
stored fp16 as DUAL COPIES [X | X<<1elem] so every dynamic window shift
resolves to a 4B-aligned base, keeping 2x engaged. dd=+-1 offset pairs
share one bias (dd^2 symmetric) and are fused per-slot via a stride-2
AP dim with a stride-0 broadcast A-side (both measured at full 2x).
Layout: partition p = 16*h_blk + w_blk is a (4h x 8w) block with halos;
only dh >= 0 is ever read, so the top h-halo is trimmed (7 stored rows).

Per-core slots (SPMD; offsets/biases are per-core register+bias data):
3 dd=+-1 pair slots (6 offsets) + 3 dd=0 single slots + 1/8 of the lone
(0,0,1) offset (spatially split mini-slot) + 1/8 of the G-pass.
"""

import numpy as np

# problem constants (hardcoded per contract)
H, W, D = 64, 64, 32
SXY, SIMG = 5.0, 0.1
RH, RW, RD = 3, 3, 1
NCORES = 8
NPAIR, NSING = 3, 3
BH, BW = 4, 8                    # central block per partition
NHB, NWB = H // BH, W // BW      # 16 x 8 blocks = 128 partitions
SH = BH + RH                     # 7 stored h rows (top halo trimmed)
SW = BW + 2 * RW                 # 14
SD = D + 2 * RD                  # 34
FREE = SH * SW * SD              # 3332 stored elems per partition
ROW = BW * SD                    # 272: fused (w,d) run per h row
PAD = 8
JP = 2 * FREE + PAD              # dual-copy tile free length (6672, even)
CENT = RW * SD + RD              # 103 (odd) central base in copy-0
CENT1 = FREE + CENT - 1          # 3434 (even) central base in copy-1
MROW = BH * ROW // NCORES        # 136: mini/G slice length per core
NCOLS = 12                       # E sums (8) | Sfinal | Smini | G | spare
NMETA = 136 + 7 + 9              # t3 slice | biases | int offsets
SQ = float(np.sqrt(0.5) / SIMG)  # sqrt(50)
S2 = float(0.5 / SIMG ** 2)      # 50
BIG = 28.0                       # halo marker: max |d|=BIG+6 -> q<=57800
                                 # finite in fp16, exp(-q)=0
NEG = -1.0e4
DENOM = float(H * W * D)
OFF_MAX = FREE + (RH * SW * SD + 2 * RW * SD + 2 * RD) + BH * ROW


def _pair_single_slots():
    """(pairs, singles, mini): pairs=(dh,dw) with dd=+-1, singles=(dh,dw)
    with dd=0, mini=(0,0,1)."""
    pairs, singles = [], []
    for dh in range(0, RH + 1):
        for dw in range(-RW, RW + 1):
            if (dh > 0) or (dh == 0 and dw > 0):
                pairs.append((dh, dw))
                singles.append((dh, dw))
    assert len(pairs) == 24 and len(singles) == 24
    return pairs, singles


def _pack(v, pad_val):
    """(H, W, D) -> [128, FREE]: per-partition block + trimmed halos."""
    vp = np.pad(v.astype(np.float32), ((RH, RH), (RW, RW), (RD, RD)),
                constant_values=pad_val)
    out = np.empty((128, SH, SW, SD), np.float32)
    for hb in range(NHB):
        for wb in range(NWB):
            out[hb * NWB + wb] = vp[hb * BH + RH:hb * BH + RH + SH,
                                    wb * BW:wb * BW + SW, :]
    return out.reshape(128, FREE)


def _dual_f16(flat, tail):
    """[128, FREE] -> fp16 [128, 2*FREE] = [X | X shifted left one elem]."""
    x = flat.astype(np.float16)
    x1 = np.empty_like(x)
    x1[:, :-1] = x[:, 1:]
    x1[:, -1] = tail
    return np.concatenate([x, x1], axis=1)


def _build_nc():
    import concourse.bass as bass
    import concourse.bacc as bacc
    import concourse.mybir as mybir
    from concourse.tile import TileContext

    f32, f16, i32 = mybir.dt.float32, mybir.dt.float16, mybir.dt.int32
    AF = mybir.ActivationFunctionType
    OP = mybir.AluOpType
    ET = mybir.EngineType

    nc = bacc.Bacc("TRN2", target_bir_lowering=False, debug=False)
    jp1 = nc.dram_tensor("jp1", [128, FREE], f16, kind="ExternalInput")
    jp0 = nc.dram_tensor("jp0", [128, FREE], f16, kind="ExternalInput")
    ap1 = nc.dram_tensor("ap1", [128, FREE], f16, kind="ExternalInput")
    ap0 = nc.dram_tensor("ap0", [128, FREE], f16, kind="ExternalInput")
    meta = nc.dram_tensor("meta", [128, NMETA], f32, kind="ExternalInput")
    out = nc.dram_tensor("out", [128, NCOLS], f32, kind="ExternalOutput")

    # patterns: [partition][(pair)][h][flat (w,d) row]
    P1 = [[JP, 128], [SW * SD, BH], [1, ROW]]                 # single B-view
    P2 = [[JP, 128], [2, 2], [SW * SD, BH], [1, ROW]]         # dd=+-1 pair
    P2B = [[JP, 128], [0, 2], [SW * SD, BH], [1, ROW]]        # bcast A-side
    PM = [[JP, 128], [1, MROW]]                               # mini slice

    with TileContext(nc) as tc:
        with tc.tile_pool(name="pers", bufs=1) as pers, \
             tc.tile_pool(name="dp2", bufs=2) as dp2, \
             tc.tile_pool(name="qp2", bufs=2) as qp2, \
             tc.tile_pool(name="ep2", bufs=2) as ep2, \
             tc.tile_pool(name="vp2", bufs=2) as vp2, \
             tc.tile_pool(name="dp1", bufs=2) as dp1, \
             tc.tile_pool(name="qp1", bufs=2) as qp1, \
             tc.tile_pool(name="ep1", bufs=2) as ep1, \
             tc.tile_pool(name="vp1", bufs=2) as vp1, \
             tc.tile_pool(name="gp", bufs=1) as gp:
            J = pers.tile([128, JP], f16, tag="J")
            A = pers.tile([128, JP], f16, tag="A")
            metatile = pers.tile([128, NMETA], f32, tag="meta")
            acc = pers.tile([128, NCOLS], f32, tag="acc")
            Q2 = pers.tile([128, 2, BH, ROW], f16, tag="Q2")
            fin = pers.tile([128, 2, BH, ROW], f16, tag="fin")

            nc.vector.memset(acc[:], 0.0)
            nc.vector.memset(Q2[:], 0.0)
            nc.vector.memset(J[:, 2 * FREE:], 0.0)
            nc.vector.memset(A[:, 2 * FREE:], 0.0)
            nc.sync.dma_start(metatile[:], meta[:])
            nc.sync.dma_start(J[:, FREE:2 * FREE], jp1[:])
            nc.sync.dma_start(A[:, FREE:2 * FREE], ap1[:])
            nc.sync.dma_start(J[:, 0:FREE], jp0[:])
            nc.sync.dma_start(A[:, 0:FREE], ap0[:])

            t3v = metatile[:, 0:MROW]
            BIAS0 = MROW
            biasv = metatile[:, BIAS0:BIAS0 + 7]   # pair0..2, sing0..2, mini
            INT0 = BIAS0 + 7
            # ints: pair joffs 0..2 | single joffs 3..5 | mini jB 6 |
            #       mini jA 7 | gjoff 8
            _, sv = nc.values_load_multi_w_load_instructions(
                metatile[0:1, INT0 + 3:INT0 + 6].bitcast(i32),
                engines=(ET.DVE,), min_val=FREE, max_val=OFF_MAX,
                skip_runtime_bounds_check=True)
            _, pv = nc.values_load_multi_w_load_instructions(
                metatile[0:1, INT0:INT0 + 3].bitcast(i32),
                engines=(ET.DVE,), min_val=0,
                max_val=RH * SW * SD + 2 * RW * SD,
                skip_runtime_bounds_check=True)
            _, mv = nc.values_load_multi_w_load_instructions(
                metatile[0:1, INT0 + 6:INT0 + 8].bitcast(i32),
                engines=(ET.DVE,), min_val=0, max_val=OFF_MAX,
                skip_runtime_bounds_check=True)
            gval = nc.values_load(
                metatile[0:1, INT0 + 8:INT0 + 9].bitcast(i32),
                engines=(ET.Activation,), min_val=FREE, max_val=OFF_MAX,
                skip_runtime_bounds_check=True)

            J_A1 = bass.AP(J.tensor, CENT1, P1)
            J_A2 = bass.AP(J.tensor, CENT1, P2B)
            a_A2 = bass.AP(A.tensor, CENT1, P2B)

            # ---- 3 single slots (dd=0; J1/A1 copies) ----
            for j in range(NSING):
                dt = dp1.tile([128, BH, ROW], f16, tag="d1")
                nc.vector.tensor_tensor(
                    dt[:], bass.AP(J.tensor, sv[j], P1), J_A1, OP.subtract)
                qt = qp1.tile([128, BH, ROW], f16, tag="q1")
                if j == 0:
                    nc.vector.tensor_tensor(qt[:], dt[:], dt[:], OP.mult)
                    escale = -S2
                else:
                    nc.scalar.activation(qt[:], dt[:], AF.Square, scale=SQ)
                    escale = -1.0
                et = ep1.tile([128, BH, ROW], f16, tag="e1")
                nc.scalar.activation(et[:], qt[:], AF.Exp, scale=escale,
                                     bias=biasv[:, NPAIR + j:NPAIR + j + 1],
                                     accum_out=acc[:, NPAIR + j:NPAIR + j + 1])
                vt = vp1.tile([128, BH, ROW], f16, tag="v1")
                nc.vector.tensor_tensor(
                    vt[:], et[:], bass.AP(A.tensor, sv[j], P1), OP.mult)
                nc.vector.tensor_tensor(Q2[:, 0], Q2[:, 0], vt[:], OP.add)

            # ---- 3 pair slots (dd=+-1; J0/A0 copies) ----
            for j in range(NPAIR):
                dt = dp2.tile([128, 2, BH, ROW], f16, tag="d2")
                nc.vector.tensor_tensor(
                    dt[:], bass.AP(J.tensor, pv[j], P2), J_A2, OP.subtract)
                qt = qp2.tile([128, 2, BH, ROW], f16, tag="q2")
                nc.scalar.activation(qt[:], dt[:], AF.Square, scale=SQ)
                et = ep2.tile([128, 2, BH, ROW], f16, tag="e2")
                nc.scalar.activation(et[:], qt[:], AF.Exp, scale=-1.0,
                                     bias=biasv[:, j:j + 1],
                                     accum_out=acc[:, j:j + 1])
                vt = vp2.tile([128, 2, BH, ROW], f16, tag="v2")
                nc.vector.tensor_tensor(
                    vt[:], et[:], bass.AP(A.tensor, pv[j], P2), OP.mult)
                nc.vector.tensor_tensor(Q2[:], Q2[:], vt[:], OP.add)

            # ---- mini slot: 1/8 of the lone (0,0,1) offset ----
            md = gp.tile([128, MROW], f16, tag="md")
            nc.vector.tensor_tensor(
                md[:], bass.AP(J.tensor, mv[0], PM),
                bass.AP(J.tensor, mv[1], PM), OP.subtract)
            mq = gp.tile([128, MROW], f16, tag="mq")
            nc.vector.tensor_tensor(mq[:], md[:], md[:], OP.mult)
            me = gp.tile([128, MROW], f16, tag="me")
            nc.scalar.activation(me[:], mq[:], AF.Exp, scale=-S2,
                                 bias=biasv[:, 6:7],
                                 accum_out=acc[:, 6:7])
            mvt = gp.tile([128, MROW], f16, tag="mv")
            nc.vector.tensor_tensor(
                mvt[:], me[:], bass.AP(A.tensor, mv[0], PM), OP.mult)
            ms = gp.tile([128, MROW], f16, tag="ms")
            nc.vector.scalar_tensor_tensor(
                ms[:], mvt[:], 1.0, bass.AP(A.tensor, mv[1], PM),
                OP.mult, OP.mult, accum_out=acc[:, 9:10])

            # ---- G-pass: 1/8 of central voxels per core ----
            qg = gp.tile([128, MROW], f16, tag="qg")
            nc.scalar.activation(
                qg[:], bass.AP(J.tensor, gval, PM), AF.Square, scale=SQ)
            ag = gp.tile([128, MROW], f32, tag="ag")
            nc.vector.scalar_tensor_tensor(
                ag[:], qg[:], -1.0, t3v[:, 0:MROW], OP.mult, OP.add)
            eg = gp.tile([128, MROW], f16, tag="eg")
            nc.scalar.activation(eg[:], ag[:], AF.Exp,
                                 accum_out=acc[:, 10:11])

            # ---- final: col8 = sum aA * (Q2 lane0 + lane1) ----
            nc.vector.tensor_tensor(Q2[:, 0], Q2[:, 0], Q2[:, 1], OP.add)
            a_A1 = bass.AP(A.tensor, CENT1, P1)
            nc.vector.scalar_tensor_tensor(
                fin[:, 0], Q2[:, 0], 1.0, a_A1, OP.mult, OP.mult,
                accum_out=acc[:, 8:9])

            nc.sync.dma_start(out[:], acc[:])
    nc.compile()
    return nc


def _host_tables(sample, spacing):
    """Per-core meta arrays."""
    sp = np.asarray(spacing, dtype=np.float64)[:, 0]
    pairs, singles = _pair_single_slots()

    # t3 = ln(noob) - 0.5*msq_center (NEG where noob == 0), central packing
    h = np.arange(H)[:, None, None]
    w = np.arange(W)[None, :, None]
    d = np.arange(D)[None, None, :]
    msq_c = ((sp[0] * h) ** 2 + (sp[1] * w) ** 2 + (sp[2] * d) ** 2) / SXY ** 2
    cnt = ((np.minimum(h, RH) + np.minimum(H - 1 - h, RH) + 1)
           * (np.minimum(w, RW) + np.minimum(W - 1 - w, RW) + 1)
           * (np.minimum(d, RD) + np.minimum(D - 1 - d, RD) + 1))
    noob = (2 * RH + 1) * (2 * RW + 1) * (2 * RD + 1) - cnt
    t3full = np.where(noob > 0, np.log(np.maximum(noob, 1)) - 0.5 * msq_c, NEG)
    t3b = np.empty((128, BH, BW, D), np.float32)
    for hb in range(NHB):
        for wb in range(NWB):
            t3b[hb * NWB + wb] = t3full[hb * BH:(hb + 1) * BH,
                                        wb * BW:(wb + 1) * BW, :]
    t3flat = t3b.reshape(128, BH * BW * D)

    def bias_of(dh, dw, dd):
        msq = ((sp[0] * dh) ** 2 + (sp[1] * dw) ** 2
               + (sp[2] * dd) ** 2) / SXY ** 2
        return -0.5 * msq

    metas = []
    for c in range(NCORES):
        m = np.zeros((128, NMETA), np.float32)
        # G/mini slice offset within the central [BH*ROW] region:
        # slice s covers h-row s//2, cols (s%2)*MROW of the flat row
        sl = (c // 2) * SW * SD + (c % 2) * MROW
        # t3 slice: central flat [BH, BW*D] per-partition; core c gets
        # flat cols [c*MROWC : ...] of the packed central block where
        # MROWC = 128 central elems. t3 is indexed over REAL d (32), the
        # on-chip G input includes d-halo cols, so slice with halos:
        # build t3 slice aligned to the on-chip [MROW] layout (d incl halo)
        t3s = np.full((128, MROW), NEG, np.float32)
        # on-chip mini/G slice = stored flat [sl+CENT .. +MROW) of copy-1
        # = rows of (w,d-with-halo); map each position to central t3 value
        r0 = (c // 2)                     # h row
        base = (c % 2) * MROW             # col offset within the 272-row
        for i in range(MROW):
            fl = base + i                 # position in the [ROW] flat row
            wcol, dcol = divmod(fl, SD)
            if 1 <= dcol <= D:
                t3s[:, i] = t3flat[:, (r0 * BW + wcol) * D + (dcol - 1)]
        m[:, 0:MROW] = t3s

        B0 = MROW
        joff_p = np.zeros(3, np.int32)
        joff_s = np.zeros(3, np.int32)
        for j in range(NPAIR):
            dh, dw = pairs[3 * c + j]
            j0 = dh * SW * SD + (RW + dw) * SD + (RD - 1)   # dd=-1 lane
            assert j0 % 2 == 0
            joff_p[j] = j0
            m[:, B0 + j] = bias_of(dh, dw, 1)
        for j in range(NSING):
            dh, dw = singles[3 * c + j]
            j0 = dh * SW * SD + (RW + dw) * SD + RD         # dd=0
            assert j0 % 2 == 1
            joff_s[j] = FREE + j0 - 1
            m[:, B0 + NPAIR + j] = bias_of(dh, dw, 0)
        m[:, B0 + 6] = bias_of(0, 0, 1)                     # mini
        I0 = B0 + 7
        m[0, I0:I0 + 3] = joff_p.view(np.float32)
        m[0, I0 + 3:I0 + 6] = joff_s.view(np.float32)
        # mini: B = central+1 (dd=+1), A = central; both shifted by slice
        mb = CENT + 1 + sl                                  # even (J0/A0)
        ma = CENT1 + sl                                     # even (J1/A1)
        assert mb % 2 == 0 and ma % 2 == 0
        m[0, I0 + 6] = np.int32(mb).view(np.float32)
        m[0, I0 + 7] = np.int32(ma).view(np.float32)
        m[0, I0 + 8] = np.int32(CENT1 + sl).view(np.float32)  # gjoff
        metas.append(m)
    return metas


def _host_inputs(y_hat_softmax, sample, spacing):
    y0 = np.asarray(y_hat_softmax, dtype=np.float32)[0, 0]
    I = np.asarray(sample, dtype=np.float32)[0, 0]
    jd = _dual_f16(_pack(I, BIG), 0.0)
    ad = _dual_f16(_pack(1.0 - 2.0 * y0, 1.0), 1.0)
    metas = _host_tables(sample, spacing)
    return [{"jp0": jd[:, :FREE], "jp1": jd[:, FREE:],
             "ap0": ad[:, :FREE], "ap1": ad[:, FREE:],
             "meta": metas[c]} for c in range(NCORES)]


def kernel(y_hat_softmax, sample, spacing):
    from concourse.bass_utils import run_bass_kernel_spmd

    in_maps = _host_inputs(y_hat_softmax, sample, spacing)
    nc = _build_nc()
    res = run_bass_kernel_spmd(nc, in_maps, core_ids=list(range(NCORES)))
    total = 0.0
    for r in res.results:
        o = r["out"].astype(np.float64)
        # cols 0..6 = sum E per slot (pairs, singles, mini); col8 = final
        # sum aA*Q2; col9 = mini product sum; col10 = G
        total += (o[:, 0:7].sum() - o[:, 8].sum() - o[:, 9].sum()
                  + o[:, 10].sum())
    return np.array(total / DENOM, dtype=np.float32)


if __name__ == "__main__":
    rng = np.random.default_rng(0)
    logits = rng.standard_normal((1, 2, H, W, D)).astype(np.float32)
    e = np.exp(logits - logits.max(axis=1, keepdims=True))
    yh = (e / e.sum(axis=1, keepdims=True)).astype(np.float32)
    smp = rng.standard_normal((1, 1, H, W, D)).astype(np.float32)
    spc = rng.uniform(0.5, 2.0, (3, 1)).astype(np.float32)
    print(kernel(yh, smp, spc))
